# revision 14
# baseline (speedup 1.0000x reference)
"""Trainium2 Bass kernel for nn_BestModel5 (dual-GRU encoder + BxB pair classifier).

Sharding (8 cores): cores 0-3 query-GRU batch shards of 64; cores 4-7 reply-GRU.
Classifier sharded 8-way over the 256 query rows (32 i-rows/core).

GRU strategy: input-projection preactivations (Wx@x + b [+mask]) are computed by
the PE directly INTO PSUM in 4-step windows; the recurrent Wh@h matmuls then
accumulate in-place and the sigmoid/tanh read PSUM directly. This removes the
two DVE adds per step from the critical chain. 1-z is computed as sigmoid(-pre)
(ACT scale=-1) so the blend is h' = z*h + (1-z)*c with only two DVE ops after
tanh. The windowed precompute doubles as PE-warmth filler (HAM K=8/8).
"""

import numpy as np
import ml_dtypes

BF16 = ml_dtypes.bfloat16

V, E, H, B, T = 100000, 256, 256, 256, 40
D_HID, D_OUT = 256, 2
NCORES = 8
BSH = 64          # batch rows per GRU shard
BT = BSH * T      # 2560 columns of xembT per core
IBLK = B // NCORES  # 32 classifier i-rows per core
WIN = 4           # GRU steps per PSUM-resident preactivation window
NWIN = T // WIN   # 10

_cache = {}


def _build():
    """Build + compile the SPMD Bass program once."""
    import concourse.bacc as bacc
    import concourse.tile as tile
    import concourse.mybir as mybir

    f32 = mybir.dt.float32
    bf16 = mybir.dt.bfloat16
    AF = mybir.ActivationFunctionType

    nc = bacc.Bacc("TRN2", target_bir_lowering=False, debug=False, num_devices=NCORES)

    def din(name, shape, dt):
        return nc.dram_tensor(name, shape, dt, kind="ExternalInput").ap()

    # per-core inputs (content differs per core; shapes identical)
    xembT = din("xembT", [E + 2, BT], bf16)      # rows 0-255 emb dims, 256 ones, 257 mask
    whg = din("whg", [H, 2 * H], bf16)           # Wg[E:E+H, :]
    wxg = din("wxg", [E, 2 * H], bf16)           # Wg[:E, :]
    wch = din("wch", [H, H], bf16)               # Wc[E:E+H, :]
    wxc = din("wxc", [E, H], bf16)               # Wc[:E, :]
    wxe = din("wxe", [2, 2 * H + H], bf16)       # row0: [bg|bc]; row1: [0|30(z)|0]
    w1q = din("w1q", [H, D_HID], bf16)           # W1[:256]
    w1r = din("w1r", [H, D_HID], bf16)           # W1[257:513]
    wdt = din("wdt", [1, IBLK // 2 * D_HID], bf16)  # W1[256] tiled 16x
    rhsb = din("rhsb", [4, IBLK * B], bf16)      # [0;ones|0;0|0;0;ones] pattern
    b1 = din("b1", [D_HID], f32)
    w2 = din("w2", [D_HID, D_OUT], bf16)
    b2w = din("b2w", [D_OUT, 2 * B], f32)        # b2 broadcast for drain-adds
    ident = din("ident", [128, 128], bf16)       # identity for PSUM-accumulate copies

    out = nc.dram_tensor("out", [D_OUT, IBLK * B], f32, kind="ExternalOutput").ap()

    with tile.TileContext(nc) as tc:
        with (
            tc.tile_pool(name="persist", bufs=1) as pp,
            tc.tile_pool(name="dram", bufs=1, space="DRAM") as dramp,
        ):
            # ---- load weights/inputs to SBUF; precompute deps first ----
            wxg_s = [pp.tile([128, 2 * H], bf16, tag=f"wxg{k}", name=f"wxg{k}") for k in range(2)]
            nc.sync.dma_start(wxg_s[0][:], wxg[0:128, :])
            nc.scalar.dma_start(wxg_s[1][:], wxg[128:256, :])
            wxc_s = [pp.tile([128, H], bf16, tag=f"wxc{k}", name=f"wxc{k}") for k in range(2)]
            nc.gpsimd.dma_start(wxc_s[0][:], wxc[0:128, :])
            nc.gpsimd.dma_start(wxc_s[1][:], wxc[128:256, :])
            wxe_s = pp.tile([2, 3 * H], bf16, tag="wxe", name="wxe")
            nc.sync.dma_start(wxe_s[:], wxe[:])

            xT = [pp.tile([128, BT], bf16, tag=f"xT{k}", name=f"xT{k}") for k in range(2)]
            xEx = pp.tile([2, BT], bf16, tag="xEx", name="xEx")
            nc.scalar.dma_start(xEx[:], xembT[256:258, :])
            nc.sync.dma_start(xT[0][:, 0:1280], xembT[0:128, 0:1280])
            nc.scalar.dma_start(xT[1][:, 0:1280], xembT[128:256, 0:1280])
            nc.gpsimd.dma_start(xT[0][:, 1280:BT], xembT[0:128, 1280:BT])
            nc.sync.dma_start(xT[1][:, 1280:BT], xembT[128:256, 1280:BT])

            whg_s = [pp.tile([128, 2 * H], bf16, tag=f"whg{k}", name=f"whg{k}") for k in range(2)]
            nc.scalar.dma_start(whg_s[0][:], whg[0:128, :])
            nc.gpsimd.dma_start(whg_s[1][:], whg[128:256, :])
            wch_s = [pp.tile([128, H], bf16, tag=f"wch{k}", name=f"wch{k}") for k in range(2)]
            nc.sync.dma_start(wch_s[0][:], wch[0:128, :])
            nc.sync.dma_start(wch_s[1][:], wch[128:256, :])

            # classifier weights (needed ~100us later; low urgency)
            w1q_s = [pp.tile([128, D_HID], bf16, tag=f"w1q{k}", name=f"w1q{k}") for k in range(2)]
            nc.gpsimd.dma_start(w1q_s[0][:], w1q[0:128, :])
            nc.gpsimd.dma_start(w1q_s[1][:], w1q[128:256, :])
            w1r_s = [pp.tile([128, D_HID], bf16, tag=f"w1r{k}", name=f"w1r{k}") for k in range(2)]
            nc.scalar.dma_start(w1r_s[0][:], w1r[0:128, :])
            nc.scalar.dma_start(w1r_s[1][:], w1r[128:256, :])
            b1_s = pp.tile([128, 2], f32, tag="b1", name="b1")
            nc.sync.dma_start(b1_s[:], b1.rearrange("(m p) -> p m", p=128))
            w2_s = [pp.tile([128, D_OUT], bf16, tag=f"w2{k}", name=f"w2{k}") for k in range(2)]
            nc.sync.dma_start(w2_s[0][:], w2[0:128, :])
            nc.sync.dma_start(w2_s[1][:], w2[128:256, :])
            b2w_s = pp.tile([D_OUT, 2 * B], f32, tag="b2w", name="b2w")
            nc.sync.dma_start(b2w_s[:], b2w[:])
            id_s = pp.tile([128, 128], bf16, tag="ident", name="ident")
            nc.scalar.dma_start(id_s[:], ident[:])

            lhs4 = pp.tile([4, IBLK // 2 * D_HID], bf16, tag="lhs4", name="lhs4")
            nc.gpsimd.dma_start(lhs4[0:1, :], wdt[:])
            nc.gpsimd.dma_start(lhs4[2:3, :], wdt[:])
            rhs4 = pp.tile([4, IBLK * B], bf16, tag="rhs4", name="rhs4")
            nc.gpsimd.dma_start(rhs4[:], rhsb[:])

            # ---- GRU: windowed PSUM-resident preactivations ----
            h_bf = None
            h_f = None
            with (
                tc.tile_pool(name="gwin", bufs=2, space="PSUM") as gwp,
                tc.tile_pool(name="cwin", bufs=2, space="PSUM") as cwp,
                tc.tile_pool(name="warm", bufs=1, space="PSUM") as wmp,
                tc.tile_pool(name="step", bufs=2) as sp,
            ):
                # HAM warm-up: ~6us of gapless junk matmuls flips the PE clock
                # gate to 8/8 before the recurrence starts.
                warm_ps = wmp.tile([128, 512], f32, tag="warm", name="warm")
                with tc.high_priority():
                    for _ in range(14):
                        nc.tensor.matmul(warm_ps[:], wxg_s[0][:, 0:128],
                                         wxg_s[0][:], start=True, stop=True,
                                         skip_group_check=True)
                warm_sb = pp.tile([1, 4], f32, tag="warmsb", name="warmsb")
                nc.vector.tensor_copy(warm_sb[:], warm_ps[0:1, 0:4])
                warm_dram = dramp.tile([1, 4], f32, tag="warmd", name="warmd")
                nc.sync.dma_start(warm_dram[:], warm_sb[:])
                gws, cws = {}, {}

                def precompute(w):
                    """Wx @ x + b (+30*mask on z) for steps [4w, 4w+4) -> PSUM."""
                    gw = gwp.tile([128, WIN * 4 * BSH], f32, tag="gw", name=f"gw{w}")
                    cw = cwp.tile([128, WIN * 2 * BSH], f32, tag="cw", name=f"cw{w}")
                    gws[w], cws[w] = gw, cw
                    cs = slice(256 * w, 256 * w + 256)
                    # start=True clears has_written for the WHOLE bank -> only
                    # the first matmul touching each bank may set it.
                    for m in range(4):
                        o = gw[:, 256 * m:256 * m + 256]
                        nc.tensor.matmul(o, wxg_s[0][:, 128 * m:128 * m + 128],
                                         xT[0][:, cs], start=(m % 2 == 0),
                                         stop=False, skip_group_check=True)
                        nc.tensor.matmul(o, wxg_s[1][:, 128 * m:128 * m + 128],
                                         xT[1][:, cs], start=False, stop=False,
                                         skip_group_check=True)
                        nc.tensor.matmul(o, wxe_s[:, 128 * m:128 * m + 128],
                                         xEx[:, cs], start=False, stop=(w == 0),
                                         skip_group_check=True)
                    for m in range(2):
                        o = cw[:, 256 * m:256 * m + 256]
                        nc.tensor.matmul(o, wxc_s[0][:, 128 * m:128 * m + 128],
                                         xT[0][:, cs], start=(m == 0),
                                         stop=False, skip_group_check=True)
                        nc.tensor.matmul(o, wxc_s[1][:, 128 * m:128 * m + 128],
                                         xT[1][:, cs], start=False, stop=False,
                                         skip_group_check=True)
                        nc.tensor.matmul(o, wxe_s[:, 512 + 128 * m:512 + 128 * m + 128],
                                         xEx[:, cs], start=False, stop=(w == 0),
                                         skip_group_check=True)

                precompute(0)
                precompute(1)

                for t in range(T):
                    w, tp = divmod(t, WIN)
                    gw, cw = gws[w], cws[w]
                    gv = gw[:].rearrange("p (m tt b) -> p m tt b", m=4, tt=WIN, b=BSH)
                    cv = cw[:].rearrange("p (m tt b) -> p m tt b", m=2, tt=WIN, b=BSH)

                    # step instructions outrank the precompute fillers so the
                    # recurrence's critical matmuls never queue behind them
                    with tc.high_priority(offset=50000):
                        # r-gate recurrent matmuls (critical path head)
                        if t > 0:
                            for m in (0, 1):
                                for k in (0, 1):
                                    nc.tensor.matmul(
                                        gv[:, m, tp, :],
                                        whg_s[k][:, 128 * m:128 * m + 128],
                                        h_bf[:, 64 * k:64 * k + 64],
                                        start=False, stop=(k == 1),
                                        skip_group_check=True)
                            r_sb = sp.tile([128, 128], f32, tag="r", name="r")
                            nc.scalar.activation(
                                r_sb[:].rearrange("p (m b) -> p m b", m=2, b=BSH),
                                gv[:, 0:2, tp, :], AF.Sigmoid)
                            rh_bf = sp.tile([128, 128], bf16, tag="rh", name="rh")
                            nc.vector.tensor_mul(rh_bf[:], r_sb[:], h_f[:])
                            # candidate matmuls (critical) ahead of z matmuls
                            for m in (0, 1):
                                for k in (0, 1):
                                    nc.tensor.matmul(
                                        cv[:, m, tp, :],
                                        wch_s[k][:, 128 * m:128 * m + 128],
                                        rh_bf[:, 64 * k:64 * k + 64],
                                        start=False, stop=(k == 1),
                                        skip_group_check=True)
                            # z-gate matmuls fill the PE gap while sigmoids run
                            for m in (2, 3):
                                for k in (0, 1):
                                    nc.tensor.matmul(
                                        gv[:, m, tp, :],
                                        whg_s[k][:, 128 * m:128 * m + 128],
                                        h_bf[:, 64 * k:64 * k + 64],
                                        start=False, stop=(k == 1),
                                        skip_group_check=True)

                        c_sb = sp.tile([128, 128], f32, tag="c", name="c")
                        nc.scalar.activation(
                            c_sb[:].rearrange("p (m b) -> p m b", m=2, b=BSH),
                            cv[:, :, tp, :], AF.Tanh)

                        # z path (off critical chain)
                        omz_sb = sp.tile([128, 128], f32, tag="omz", name="omz")
                        nc.scalar.activation(
                            omz_sb[:].rearrange("p (m b) -> p m b", m=2, b=BSH),
                            gv[:, 2:4, tp, :], AF.Sigmoid, scale=-1.0)
                        if t > 0:
                            z_sb = sp.tile([128, 128], f32, tag="z", name="z")
                            nc.scalar.activation(
                                z_sb[:].rearrange("p (m b) -> p m b", m=2, b=BSH),
                                gv[:, 2:4, tp, :], AF.Sigmoid)
                            zh = sp.tile([128, 128], f32, tag="zh", name="zh")
                            nc.vector.tensor_mul(zh[:], z_sb[:], h_f[:])

                        omzc = sp.tile([128, 128], f32, tag="omzc", name="omzc")
                        nc.vector.tensor_mul(omzc[:], omz_sb[:], c_sb[:])

                        h_bf = pp.tile([128, 128], bf16, tag="hbf", name="hbf", bufs=2)
                        if t > 0:
                            nc.vector.tensor_add(h_bf[:], zh[:], omzc[:])
                        else:
                            nc.vector.tensor_copy(h_bf[:], omzc[:])
                        if t < T - 1:
                            h_f_new = pp.tile([128, 128], f32, tag="hf", name="hf", bufs=2)
                            if t > 0:
                                nc.gpsimd.tensor_add(h_f_new[:], zh[:], omzc[:])
                            else:
                                nc.gpsimd.tensor_copy(h_f_new[:], omzc[:])
                            h_f = h_f_new

                    if tp == 1 and w + 2 < NWIN:
                        precompute(w + 2)

            # ---- exchange encodings ----
            ag_in = dramp.tile([128, 128], bf16, tag="agin", name="agin")
            ag_out = dramp.tile([NCORES, 128, 128], bf16, tag="agout", name="agout")

            nc.sync.dma_start(ag_in[:], h_bf[:])
            nc.gpsimd.collective_compute(
                "AllGather", mybir.AluOpType.bypass,
                replica_groups=[list(range(NCORES))],
                ins=[ag_in.opt()], outs=[ag_out.opt()])

            # per-core q slice: rows [32*core, 32*core+32) live on gathered
            # block core//2, batch-half core%2 -> one dynamic-offset DMA
            qloc = pp.tile([128, 2 * IBLK], bf16, tag="qloc", name="qloc")  # [p, c*32+b]
            pid = nc.scalar.partition_id()
            src_v = ag_out[:].rearrange("n p (c h b) -> n p c h b", c=2, h=2, b=32)
            nc.scalar.dma_start(
                qloc[:].rearrange("p (c b) -> p c b", c=2, b=32),
                src_v[pid // 2, :, :, pid % 2, :])
            rT = pp.tile([128, 2 * B], bf16, tag="rT", name="rT")  # [p, c*256+64k+b]
            for c in range(2):
                for k in range(4):
                    eng = (nc.gpsimd, nc.sync)[k % 2]
                    eng.dma_start(
                        rT[:, 256 * c + 64 * k:256 * c + 64 * k + 64],
                        ag_out[4 + k, :, 64 * c:64 * c + 64])

            # ---- classifier ----
            with (
                tc.tile_pool(name="spsum", bufs=2, space="PSUM") as sps,
                tc.tile_pool(name="hpsum", bufs=2, space="PSUM") as hps,
                tc.tile_pool(name="lpsum", bufs=2, space="PSUM") as lps,
                tc.tile_pool(name="cls", bufs=3) as cp,
            ):
                # R1T + b1: [128, m*256 + j] f32 (does not need qloc)
                ps3 = sps.tile([128, 2 * B], f32, tag="sps", name="sps")
                for m in range(2):
                    for k in range(2):
                        nc.tensor.matmul(ps3[:, 256 * m:256 * m + 256],
                                         w1r_s[k][:, 128 * m:128 * m + 128],
                                         rT[:, 256 * k:256 * k + 256],
                                         start=(k == 0), stop=(k == 1))
                r1tb = pp.tile([128, 2 * B], f32, tag="r1tb", name="r1tb")
                for m in range(2):
                    nc.scalar.activation(r1tb[:, 256 * m:256 * m + 256],
                                         ps3[:, 256 * m:256 * m + 256],
                                         AF.Identity, bias=b1_s[:, m:m + 1])
                r1tb2 = pp.tile([128, 4 * B], bf16, tag="r1tb2", name="r1tb2")
                r2v = r1tb2[:].rearrange("p (m ii j) -> p m ii j", m=2, ii=2, j=B)
                for ii in range(2):
                    nc.vector.tensor_copy(
                        r2v[:, :, ii, :],
                        r1tb[:].rearrange("p (m j) -> p m j", m=2, j=B))

                # Q1 rows for my i's: [32, 256] bf16
                ps = sps.tile([IBLK, D_HID], f32, tag="sps", name="sps")
                for c in range(2):
                    nc.tensor.matmul(ps[:], qloc[:, 32 * c:32 * c + 32],
                                     w1q_s[c][:], start=(c == 0), stop=(c == 1))
                q1 = pp.tile([IBLK, D_HID], bf16, tag="q1", name="q1")
                nc.scalar.activation(q1[:], ps[:], AF.Copy, bias=0.0)
                nc.sync.dma_start(lhs4[1:2, :], q1[0:16, :])
                nc.sync.dma_start(lhs4[3:4, :], q1[16:32, :])

                # dist rows for my i's: [32, 256] bf16
                ps2 = sps.tile([IBLK, B], f32, tag="sps", name="sps")
                for c in range(2):
                    nc.tensor.matmul(ps2[:], qloc[:, 32 * c:32 * c + 32],
                                     rT[:, 256 * c:256 * c + 256],
                                     start=(c == 0), stop=(c == 1))
                dist = pp.tile([IBLK, B], bf16, tag="dist", name="dist")
                nc.scalar.activation(dist[:], ps2[:], AF.Copy, bias=0.0)
                nc.gpsimd.dma_start(
                    rhs4[0:1, :].rearrange("o (p ii j) -> o p ii j",
                                           p=IBLK // 2, ii=2, j=B)[:, :, 0, :],
                    dist[0:16, :])
                nc.gpsimd.dma_start(
                    rhs4[2:3, :].rearrange("o (p ii j) -> o p ii j",
                                           p=IBLK // 2, ii=2, j=B)[:, :, 1, :],
                    dist[16:32, :])

                out_sb = pp.tile([D_OUT, IBLK * B], f32, tag="outsb", name="outsb")
                NFOLD = 6  # prs whose r1 add is PE-folded (gelu reads PSUM)
                for pr in range(IBLK // 2):
                    # h1 pair tile: col = 512*m + 256*ii + j  (ii = i in pair)
                    fold = pr < NFOLD
                    h_ps = hps.tile([128, 4 * B], f32, tag="hps", name="hps")
                    for m in range(2):
                        nc.tensor.matmul(
                            h_ps[:, 512 * m:512 * m + 512],
                            lhs4[0:4,
                                 D_HID * pr + 128 * m:D_HID * pr + 128 * m + 128],
                            rhs4[0:4, 2 * B * pr:2 * B * pr + 2 * B],
                            start=True, stop=(not fold),
                            skip_group_check=True)
                    h1 = cp.tile([128, 4 * B], bf16, tag="h1", name="h1")
                    if fold:
                        # accumulate r1 broadcast into PSUM via identity matmul
                        for m in range(2):
                            nc.tensor.matmul(h_ps[:, 512 * m:512 * m + 512],
                                             id_s[:],
                                             r1tb2[:, 512 * m:512 * m + 512],
                                             start=False, stop=True,
                                             skip_group_check=True)
                        nc.scalar.activation(h1[:], h_ps[:], AF.Gelu_apprx_tanh)
                    else:
                        h1p = cp.tile([128, 4 * B], f32, tag="h1p", name="h1p")
                        nc.vector.tensor_add(h1p[:], h_ps[:], r1tb2[:])
                        nc.scalar.activation(h1[:], h1p[:], AF.Gelu_apprx_tanh)
                    l_ps = lps.tile([D_OUT, 2 * B], f32, tag="lps", name="lps")
                    for k in range(2):
                        nc.tensor.matmul(l_ps[:], w2_s[k][:],
                                         h1[:, 512 * k:512 * k + 512],
                                         start=(k == 0), stop=(k == 1))
                    nc.vector.tensor_add(out_sb[:, 512 * pr:512 * pr + 512],
                                         l_ps[:], b2w_s[:])
                nc.sync.dma_start(out[:], out_sb[:])

    nc.compile()
    return nc


def _rhs_base():
    """[4, IBLK*B] pattern: per 512-col pair-block rows are
    [0,0],[ones,0],[0,0],[0,ones] - dist blocks get DMA'd in on device."""
    r = np.zeros((4, IBLK * B), dtype=BF16)
    v = r.reshape(4, IBLK // 2, 2, B)
    v[1, :, 0, :] = 1.0
    v[3, :, 1, :] = 1.0
    return r


def _prep_inputs(inputs):
    """Host-side prep: embed+transpose sequences, split weights, per-core maps."""
    emb = inputs["embeddings"]
    in_maps = []
    f32 = np.float32

    # classifier tensors (identical on all cores)
    W1, b1, W2, b2 = (inputs["W1"], inputs["b1"], inputs["W2"], inputs["b2"])
    common = {
        "w1q": np.ascontiguousarray(W1[:H]).astype(BF16),
        "w1r": np.ascontiguousarray(W1[H + 1:]).astype(BF16),
        "wdt": np.tile(np.ascontiguousarray(W1[H:H + 1]).astype(BF16),
                       (1, IBLK // 2)),
        "rhsb": _rhs_base(),
        "b1": b1.astype(f32),
        "w2": W2.astype(BF16),
        "b2w": np.tile(b2.astype(f32).reshape(D_OUT, 1), (1, 2 * B)),
        "ident": np.eye(128, dtype=BF16),
    }

    for core in range(NCORES):
        enc = core // 4
        s = core % 4
        if enc == 0:
            seqs, lens = inputs["input_queries"], inputs["query_lengths"]
            Wg, bgv, Wc, bcv = (inputs["Wg_q"], inputs["bg_q"],
                                inputs["Wc_q"], inputs["bc_q"])
        else:
            seqs, lens = inputs["input_replies"], inputs["reply_lengths"]
            Wg, bgv, Wc, bcv = (inputs["Wg_r"], inputs["bg_r"],
                                inputs["Wc_r"], inputs["bc_r"])
        rows = slice(BSH * s, BSH * s + BSH)
        xe = emb[seqs[rows]]                       # [64, 40, 256]
        xT = np.transpose(xe, (2, 1, 0)).reshape(E, BT)  # col = t*64+b
        ones_row = np.ones((1, BT), f32)
        lmask = (np.arange(T)[:, None] >= lens[rows][None, :]) \
            .astype(f32).reshape(1, BT)
        xembT = np.concatenate([xT, ones_row, lmask], axis=0).astype(BF16)

        # extra-row weights: row0 multiplies ones (biases), row1 multiplies
        # the length mask (+30 on z-gate preactivation freezes h)
        wxe = np.zeros((2, 3 * H), f32)
        wxe[0, :2 * H] = bgv
        wxe[0, 2 * H:] = bcv
        wxe[1, H:2 * H] = 30.0

        m = {
            "xembT": xembT,
            "whg": np.ascontiguousarray(Wg[E:]).astype(BF16),
            "wxg": np.ascontiguousarray(Wg[:E]).astype(BF16),
            "wch": np.ascontiguousarray(Wc[E:]).astype(BF16),
            "wxc": np.ascontiguousarray(Wc[:E]).astype(BF16),
            "wxe": wxe.astype(BF16),
        }
        m.update(common)
        in_maps.append(m)
    return in_maps


def run_cores(in_maps, trace=False):
    from concourse.bass_utils import run_bass_kernel_spmd
    from concourse.bass_interp import get_hw_module

    if "nc" not in _cache:
        _cache["nc"] = _build()
    nc = _cache["nc"]
    old = nc.m
    nc.m = _cache.setdefault("hwm", get_hw_module(nc.m))
    try:
        res = run_bass_kernel_spmd(nc, in_maps, core_ids=list(range(NCORES)),
                                   trace=trace)
    finally:
        nc.m = old
    return res


def kernel(**inputs):
    in_maps = _prep_inputs(inputs)
    res = run_cores(in_maps)
    logits = np.zeros((B, B, 2), np.float32)
    for core in range(NCORES):
        o = res.results[core]["out"]               # [2, 32*256]
        # pair layout: col = 512*pr + 256*ii + j, local row = 16*ii + pr
        logits[IBLK * core:IBLK * core + IBLK] = \
            o.reshape(2, 16, 2, B).transpose(2, 1, 3, 0).reshape(IBLK, B, 2)
    pos = logits[np.arange(B), np.arange(B)]
    qi, ri = np.nonzero(~np.eye(B, dtype=bool))
    neg = logits[qi, ri]
    return np.concatenate([pos, neg], axis=0).astype(np.float32)


if __name__ == "__main__":
    _build()
    print("build OK")


# revision 17
# speedup vs baseline: 1.1264x; 1.1264x over previous
"""Trainium2 Bass kernel for nn_BestModel5 (dual-GRU encoder + BxB pair classifier).

Sharding (8 cores): cores 0-3 query-GRU batch shards of 64; cores 4-7 reply-GRU.
Classifier sharded 8-way over the 256 query rows (32 i-rows/core).

GRU strategy: input-projection preactivations (Wx@x + b [+mask]) are computed by
the PE directly INTO PSUM in 4-step windows; the recurrent Wh@h matmuls then
accumulate in-place and the sigmoid/tanh read PSUM directly. This removes the
two DVE adds per step from the critical chain. 1-z is computed as sigmoid(-pre)
(ACT scale=-1) so the blend is h' = z*h + (1-z)*c with only two DVE ops after
tanh. The windowed precompute doubles as PE-warmth filler (HAM K=8/8).
"""

import numpy as np
import ml_dtypes

BF16 = ml_dtypes.bfloat16


def _enable_ldw_opt():
    """Re-enable LDWEIGHTS pipelining (off in this env's default flags)."""
    from concourse.compiler_utils import get_compiler_flags, set_compiler_flags

    flags = [f.replace("--enable-ldw-opt=false", "--enable-ldw-opt=true")
             for f in get_compiler_flags()]
    set_compiler_flags(flags)

V, E, H, B, T = 100000, 256, 256, 256, 40
D_HID, D_OUT = 256, 2
NCORES = 8
BSH = 64          # batch rows per GRU shard
BT = BSH * T      # 2560 columns of xembT per core
IBLK = B // NCORES  # 32 classifier i-rows per core
WIN = 4           # GRU steps per PSUM-resident preactivation window
NWIN = T // WIN   # 10

_cache = {}


def _build():
    """Build + compile the SPMD Bass program once."""
    import concourse.bacc as bacc
    import concourse.tile as tile
    import concourse.mybir as mybir

    f32 = mybir.dt.float32
    bf16 = mybir.dt.bfloat16
    AF = mybir.ActivationFunctionType

    _enable_ldw_opt()
    nc = bacc.Bacc("TRN2", target_bir_lowering=False, debug=False, num_devices=NCORES)

    def din(name, shape, dt):
        return nc.dram_tensor(name, shape, dt, kind="ExternalInput").ap()

    # per-core inputs (content differs per core; shapes identical)
    xembT = din("xembT", [E + 2, BT], bf16)      # rows 0-255 emb dims, 256 ones, 257 mask
    whg = din("whg", [H, 2 * H], bf16)           # Wg[E:E+H, :]
    wxg = din("wxg", [E, 2 * H], bf16)           # Wg[:E, :]
    wch = din("wch", [H, H], bf16)               # Wc[E:E+H, :]
    wxc = din("wxc", [E, H], bf16)               # Wc[:E, :]
    wxe = din("wxe", [2, 2 * H + H], bf16)       # row0: [bg|bc]; row1: [0|30(z)|0]
    w1q = din("w1q", [H, D_HID], bf16)           # W1[:256]
    w1r = din("w1r", [H, D_HID], bf16)           # W1[257:513]
    wdt = din("wdt", [1, IBLK // 2 * D_HID], bf16)  # W1[256] tiled 16x
    rhsb = din("rhsb", [4, IBLK * B], bf16)      # [0;ones|0;0|0;0;ones] pattern
    b1 = din("b1", [D_HID], f32)
    w2 = din("w2", [D_HID, D_OUT], bf16)
    b2w = din("b2w", [D_OUT, 2 * B], f32)        # b2 broadcast for drain-adds
    ident = din("ident", [128, 128], bf16)       # identity for PSUM-accumulate copies

    out = nc.dram_tensor("out", [D_OUT, IBLK * B], f32, kind="ExternalOutput").ap()

    with tile.TileContext(nc) as tc:
        with (
            tc.tile_pool(name="persist", bufs=1) as pp,
            tc.tile_pool(name="dram", bufs=1, space="DRAM") as dramp,
        ):
            # ---- load weights/inputs to SBUF; precompute deps first ----
            wxg_s = [pp.tile([128, 2 * H], bf16, tag=f"wxg{k}", name=f"wxg{k}") for k in range(2)]
            nc.sync.dma_start(wxg_s[0][:], wxg[0:128, :])
            nc.scalar.dma_start(wxg_s[1][:], wxg[128:256, :])
            wxc_s = [pp.tile([128, H], bf16, tag=f"wxc{k}", name=f"wxc{k}") for k in range(2)]
            nc.gpsimd.dma_start(wxc_s[0][:], wxc[0:128, :])
            nc.gpsimd.dma_start(wxc_s[1][:], wxc[128:256, :])
            wxe_s = pp.tile([2, 3 * H], bf16, tag="wxe", name="wxe")
            nc.sync.dma_start(wxe_s[:], wxe[:])

            xT = [pp.tile([128, BT], bf16, tag=f"xT{k}", name=f"xT{k}") for k in range(2)]
            xEx = pp.tile([2, BT], bf16, tag="xEx", name="xEx")
            nc.scalar.dma_start(xEx[:], xembT[256:258, :])
            nc.sync.dma_start(xT[0][:, 0:1280], xembT[0:128, 0:1280])
            nc.scalar.dma_start(xT[1][:, 0:1280], xembT[128:256, 0:1280])
            nc.gpsimd.dma_start(xT[0][:, 1280:BT], xembT[0:128, 1280:BT])
            nc.sync.dma_start(xT[1][:, 1280:BT], xembT[128:256, 1280:BT])

            whg_s = [pp.tile([128, 2 * H], bf16, tag=f"whg{k}", name=f"whg{k}") for k in range(2)]
            nc.scalar.dma_start(whg_s[0][:], whg[0:128, :])
            nc.gpsimd.dma_start(whg_s[1][:], whg[128:256, :])
            wch_s = [pp.tile([128, H], bf16, tag=f"wch{k}", name=f"wch{k}") for k in range(2)]
            nc.sync.dma_start(wch_s[0][:], wch[0:128, :])
            nc.sync.dma_start(wch_s[1][:], wch[128:256, :])

            # classifier weights (needed ~100us later; low urgency)
            w1q_s = [pp.tile([128, D_HID], bf16, tag=f"w1q{k}", name=f"w1q{k}") for k in range(2)]
            nc.gpsimd.dma_start(w1q_s[0][:], w1q[0:128, :])
            nc.gpsimd.dma_start(w1q_s[1][:], w1q[128:256, :])
            w1r_s = [pp.tile([128, D_HID], bf16, tag=f"w1r{k}", name=f"w1r{k}") for k in range(2)]
            nc.scalar.dma_start(w1r_s[0][:], w1r[0:128, :])
            nc.scalar.dma_start(w1r_s[1][:], w1r[128:256, :])
            b1_s = pp.tile([128, 2], f32, tag="b1", name="b1")
            nc.sync.dma_start(b1_s[:], b1.rearrange("(m p) -> p m", p=128))
            w2_s = [pp.tile([128, D_OUT], bf16, tag=f"w2{k}", name=f"w2{k}") for k in range(2)]
            nc.sync.dma_start(w2_s[0][:], w2[0:128, :])
            nc.sync.dma_start(w2_s[1][:], w2[128:256, :])
            b2w_s = pp.tile([D_OUT, 2 * B], f32, tag="b2w", name="b2w")
            nc.sync.dma_start(b2w_s[:], b2w[:])
            id_s = pp.tile([128, 128], bf16, tag="ident", name="ident")
            nc.scalar.dma_start(id_s[:], ident[:])

            lhs4 = pp.tile([4, IBLK // 2 * D_HID], bf16, tag="lhs4", name="lhs4")
            nc.gpsimd.dma_start(lhs4[0:1, :], wdt[:])
            nc.gpsimd.dma_start(lhs4[2:3, :], wdt[:])
            rhs4 = pp.tile([4, IBLK * B], bf16, tag="rhs4", name="rhs4")
            nc.gpsimd.dma_start(rhs4[:], rhsb[:])

            # ---- GRU: windowed PSUM-resident preactivations ----
            h_bf = None
            h_f = None
            with (
                tc.tile_pool(name="gwin", bufs=3, space="PSUM") as gwp,
                tc.tile_pool(name="cwin", bufs=2, space="PSUM") as cwp,
                tc.tile_pool(name="step", bufs=2) as sp,
            ):
                gws, cws = {}, {}

                def precompute(w, first=False):
                    """Wx @ x + b (+30*mask on z) for steps [4w, 4w+4) -> PSUM.
                    Candidate part first: its pool has less slack (bufs=2)."""
                    gw = gwp.tile([128, WIN * 4 * BSH], f32, tag="gw", name=f"gw{w}")
                    cw = cwp.tile([128, WIN * 2 * BSH], f32, tag="cw", name=f"cw{w}")
                    gws[w], cws[w] = gw, cw
                    if first:
                        # HAM warm-up: gapless junk matmuls into window-0's
                        # gate bank (overwritten by the real precompute below)
                        with tc.high_priority():
                            for _ in range(14):
                                nc.tensor.matmul(gw[:, 0:512],
                                                 wxg_s[0][:, 0:128],
                                                 wxg_s[0][:], start=True,
                                                 stop=True,
                                                 skip_group_check=True)
                        warm_sb = pp.tile([1, 4], f32, tag="warmsb", name="warmsb")
                        nc.vector.tensor_copy(warm_sb[:], gw[0:1, 0:4])
                        warm_dram = dramp.tile([1, 4], f32, tag="warmd", name="warmd")
                        nc.sync.dma_start(warm_dram[:], warm_sb[:])
                    cs = slice(256 * w, 256 * w + 256)
                    # start=True clears has_written for the WHOLE bank -> only
                    # the first matmul touching each bank may set it.
                    for m in range(2):
                        o = cw[:, 256 * m:256 * m + 256]
                        nc.tensor.matmul(o, wxc_s[0][:, 128 * m:128 * m + 128],
                                         xT[0][:, cs], start=(m == 0),
                                         stop=False, skip_group_check=True)
                        nc.tensor.matmul(o, wxc_s[1][:, 128 * m:128 * m + 128],
                                         xT[1][:, cs], start=False, stop=False,
                                         skip_group_check=True)
                        nc.tensor.matmul(o, wxe_s[:, 512 + 128 * m:512 + 128 * m + 128],
                                         xEx[:, cs], start=False, stop=(w == 0),
                                         skip_group_check=True)
                    for m in range(4):
                        o = gw[:, 256 * m:256 * m + 256]
                        nc.tensor.matmul(o, wxg_s[0][:, 128 * m:128 * m + 128],
                                         xT[0][:, cs], start=(m % 2 == 0),
                                         stop=False, skip_group_check=True)
                        nc.tensor.matmul(o, wxg_s[1][:, 128 * m:128 * m + 128],
                                         xT[1][:, cs], start=False, stop=False,
                                         skip_group_check=True)
                        nc.tensor.matmul(o, wxe_s[:, 128 * m:128 * m + 128],
                                         xEx[:, cs], start=False, stop=(w == 0),
                                         skip_group_check=True)

                precompute(0, first=True)
                precompute(1)

                for t in range(T):
                    w, tp = divmod(t, WIN)
                    gw, cw = gws[w], cws[w]
                    gv = gw[:].rearrange("p (m tt b) -> p m tt b", m=4, tt=WIN, b=BSH)
                    cv = cw[:].rearrange("p (m tt b) -> p m tt b", m=2, tt=WIN, b=BSH)

                    # step instructions outrank the precompute fillers so the
                    # recurrence's critical matmuls never queue behind them
                    with tc.high_priority(offset=50000):
                        # r-gate recurrent matmuls (critical path head)
                        if t > 0:
                            for m in (0, 1):
                                for k in (0, 1):
                                    nc.tensor.matmul(
                                        gv[:, m, tp, :],
                                        whg_s[k][:, 128 * m:128 * m + 128],
                                        h_bf[:, 64 * k:64 * k + 64],
                                        start=False, stop=(k == 1),
                                        skip_group_check=True)
                            r_sb = sp.tile([128, 128], f32, tag="r", name="r")
                            nc.scalar.activation(
                                r_sb[:].rearrange("p (m b) -> p m b", m=2, b=BSH),
                                gv[:, 0:2, tp, :], AF.Sigmoid)
                            rh_bf = sp.tile([128, 128], bf16, tag="rh", name="rh")
                            nc.vector.tensor_mul(rh_bf[:], r_sb[:], h_f[:])
                            # candidate matmuls (critical) ahead of z matmuls
                            for m in (0, 1):
                                for k in (0, 1):
                                    nc.tensor.matmul(
                                        cv[:, m, tp, :],
                                        wch_s[k][:, 128 * m:128 * m + 128],
                                        rh_bf[:, 64 * k:64 * k + 64],
                                        start=False, stop=(k == 1),
                                        skip_group_check=True)
                            # z-gate matmuls fill the PE gap while sigmoids run
                            for m in (2, 3):
                                for k in (0, 1):
                                    nc.tensor.matmul(
                                        gv[:, m, tp, :],
                                        whg_s[k][:, 128 * m:128 * m + 128],
                                        h_bf[:, 64 * k:64 * k + 64],
                                        start=False, stop=(k == 1),
                                        skip_group_check=True)

                        c_sb = sp.tile([128, 128], f32, tag="c", name="c")
                        nc.scalar.activation(
                            c_sb[:].rearrange("p (m b) -> p m b", m=2, b=BSH),
                            cv[:, :, tp, :], AF.Tanh)

                        # z path (off critical chain)
                        omz_sb = sp.tile([128, 128], f32, tag="omz", name="omz")
                        nc.scalar.activation(
                            omz_sb[:].rearrange("p (m b) -> p m b", m=2, b=BSH),
                            gv[:, 2:4, tp, :], AF.Sigmoid, scale=-1.0)
                        if t > 0:
                            z_sb = sp.tile([128, 128], f32, tag="z", name="z")
                            nc.scalar.activation(
                                z_sb[:].rearrange("p (m b) -> p m b", m=2, b=BSH),
                                gv[:, 2:4, tp, :], AF.Sigmoid)
                            zh = sp.tile([128, 128], f32, tag="zh", name="zh")
                            nc.vector.tensor_mul(zh[:], z_sb[:], h_f[:])

                        omzc = sp.tile([128, 128], f32, tag="omzc", name="omzc")
                        nc.vector.tensor_mul(omzc[:], omz_sb[:], c_sb[:])

                        h_bf = pp.tile([128, 128], bf16, tag="hbf", name="hbf", bufs=2)
                        if t > 0:
                            nc.vector.tensor_add(h_bf[:], zh[:], omzc[:])
                        else:
                            nc.vector.tensor_copy(h_bf[:], omzc[:])
                        if t < T - 1:
                            h_f_new = pp.tile([128, 128], f32, tag="hf", name="hf", bufs=2)
                            if t > 0:
                                nc.gpsimd.tensor_add(h_f_new[:], zh[:], omzc[:])
                            else:
                                nc.gpsimd.tensor_copy(h_f_new[:], omzc[:])
                            h_f = h_f_new

                    if tp == 1 and w + 2 < NWIN:
                        precompute(w + 2)

            # ---- exchange encodings ----
            ag_in = dramp.tile([128, 128], bf16, tag="agin", name="agin")
            ag_out = dramp.tile([NCORES, 128, 128], bf16, tag="agout", name="agout")

            nc.sync.dma_start(ag_in[:], h_bf[:])
            nc.gpsimd.collective_compute(
                "AllGather", mybir.AluOpType.bypass,
                replica_groups=[list(range(NCORES))],
                ins=[ag_in.opt()], outs=[ag_out.opt()])

            # per-core q slice: rows [32*core, 32*core+32) live on gathered
            # block core//2, batch-half core%2 -> one dynamic-offset DMA
            qloc = pp.tile([128, 2 * IBLK], bf16, tag="qloc", name="qloc")  # [p, c*32+b]
            pid = nc.scalar.partition_id()
            src_v = ag_out[:].rearrange("n p (c h b) -> n p c h b", c=2, h=2, b=32)
            nc.scalar.dma_start(
                qloc[:].rearrange("p (c b) -> p c b", c=2, b=32),
                src_v[pid // 2, :, :, pid % 2, :])
            rT = pp.tile([128, 2 * B], bf16, tag="rT", name="rT")  # [p, c*256+64k+b]
            for c in range(2):
                for k in range(4):
                    eng = (nc.gpsimd, nc.sync)[k % 2]
                    eng.dma_start(
                        rT[:, 256 * c + 64 * k:256 * c + 64 * k + 64],
                        ag_out[4 + k, :, 64 * c:64 * c + 64])

            # ---- classifier ----
            with (
                tc.tile_pool(name="spsum", bufs=2, space="PSUM") as sps,
                tc.tile_pool(name="hpsum", bufs=2, space="PSUM") as hps,
                tc.tile_pool(name="lpsum", bufs=2, space="PSUM") as lps,
                tc.tile_pool(name="cls", bufs=3) as cp,
            ):
                # R1T + b1: [128, m*256 + j] f32 (does not need qloc)
                ps3 = sps.tile([128, 2 * B], f32, tag="sps", name="sps")
                for m in range(2):
                    for k in range(2):
                        nc.tensor.matmul(ps3[:, 256 * m:256 * m + 256],
                                         w1r_s[k][:, 128 * m:128 * m + 128],
                                         rT[:, 256 * k:256 * k + 256],
                                         start=(k == 0), stop=(k == 1))
                r1tb = pp.tile([128, 2 * B], f32, tag="r1tb", name="r1tb")
                for m in range(2):
                    nc.scalar.activation(r1tb[:, 256 * m:256 * m + 256],
                                         ps3[:, 256 * m:256 * m + 256],
                                         AF.Identity, bias=b1_s[:, m:m + 1])
                r1tb2 = pp.tile([128, 4 * B], bf16, tag="r1tb2", name="r1tb2")
                r2v = r1tb2[:].rearrange("p (m ii j) -> p m ii j", m=2, ii=2, j=B)
                for ii in range(2):
                    nc.vector.tensor_copy(
                        r2v[:, :, ii, :],
                        r1tb[:].rearrange("p (m j) -> p m j", m=2, j=B))

                # Q1 rows for my i's: [32, 256] bf16
                ps = sps.tile([IBLK, D_HID], f32, tag="sps", name="sps")
                for c in range(2):
                    nc.tensor.matmul(ps[:], qloc[:, 32 * c:32 * c + 32],
                                     w1q_s[c][:], start=(c == 0), stop=(c == 1))
                q1 = pp.tile([IBLK, D_HID], bf16, tag="q1", name="q1")
                nc.scalar.activation(q1[:], ps[:], AF.Copy, bias=0.0)
                nc.sync.dma_start(lhs4[1:2, :], q1[0:16, :])
                nc.sync.dma_start(lhs4[3:4, :], q1[16:32, :])

                # dist rows for my i's: [32, 256] bf16
                ps2 = sps.tile([IBLK, B], f32, tag="sps", name="sps")
                for c in range(2):
                    nc.tensor.matmul(ps2[:], qloc[:, 32 * c:32 * c + 32],
                                     rT[:, 256 * c:256 * c + 256],
                                     start=(c == 0), stop=(c == 1))
                dist = pp.tile([IBLK, B], bf16, tag="dist", name="dist")
                nc.scalar.activation(dist[:], ps2[:], AF.Copy, bias=0.0)
                nc.gpsimd.dma_start(
                    rhs4[0:1, :].rearrange("o (p ii j) -> o p ii j",
                                           p=IBLK // 2, ii=2, j=B)[:, :, 0, :],
                    dist[0:16, :])
                nc.gpsimd.dma_start(
                    rhs4[2:3, :].rearrange("o (p ii j) -> o p ii j",
                                           p=IBLK // 2, ii=2, j=B)[:, :, 1, :],
                    dist[16:32, :])

                out_sb = pp.tile([D_OUT, IBLK * B], f32, tag="outsb", name="outsb")
                NFOLD = 6  # prs whose r1 add is PE-folded (gelu reads PSUM)
                for pr in range(IBLK // 2):
                    # h1 pair tile: col = 512*m + 256*ii + j  (ii = i in pair)
                    fold = pr < NFOLD
                    h_ps = hps.tile([128, 4 * B], f32, tag="hps", name="hps")
                    for m in range(2):
                        nc.tensor.matmul(
                            h_ps[:, 512 * m:512 * m + 512],
                            lhs4[0:4,
                                 D_HID * pr + 128 * m:D_HID * pr + 128 * m + 128],
                            rhs4[0:4, 2 * B * pr:2 * B * pr + 2 * B],
                            start=True, stop=(not fold),
                            skip_group_check=True)
                    h1 = cp.tile([128, 4 * B], bf16, tag="h1", name="h1")
                    if fold:
                        # accumulate r1 broadcast into PSUM via identity matmul
                        for m in range(2):
                            nc.tensor.matmul(h_ps[:, 512 * m:512 * m + 512],
                                             id_s[:],
                                             r1tb2[:, 512 * m:512 * m + 512],
                                             start=False, stop=True,
                                             skip_group_check=True)
                        nc.scalar.activation(h1[:], h_ps[:], AF.Gelu_apprx_tanh)
                    else:
                        h1p = cp.tile([128, 4 * B], f32, tag="h1p", name="h1p")
                        nc.vector.tensor_add(h1p[:], h_ps[:], r1tb2[:])
                        nc.scalar.activation(h1[:], h1p[:], AF.Gelu_apprx_tanh)
                    l_ps = lps.tile([D_OUT, 2 * B], f32, tag="lps", name="lps")
                    for k in range(2):
                        nc.tensor.matmul(l_ps[:], w2_s[k][:],
                                         h1[:, 512 * k:512 * k + 512],
                                         start=(k == 0), stop=(k == 1))
                    nc.vector.tensor_add(out_sb[:, 512 * pr:512 * pr + 512],
                                         l_ps[:], b2w_s[:])
                nc.sync.dma_start(out[:], out_sb[:])

    nc.compile()
    return nc


def _rhs_base():
    """[4, IBLK*B] pattern: per 512-col pair-block rows are
    [0,0],[ones,0],[0,0],[0,ones] - dist blocks get DMA'd in on device."""
    r = np.zeros((4, IBLK * B), dtype=BF16)
    v = r.reshape(4, IBLK // 2, 2, B)
    v[1, :, 0, :] = 1.0
    v[3, :, 1, :] = 1.0
    return r


def _prep_inputs(inputs):
    """Host-side prep: embed+transpose sequences, split weights, per-core maps."""
    emb = inputs["embeddings"]
    in_maps = []
    f32 = np.float32

    # classifier tensors (identical on all cores)
    W1, b1, W2, b2 = (inputs["W1"], inputs["b1"], inputs["W2"], inputs["b2"])
    common = {
        "w1q": np.ascontiguousarray(W1[:H]).astype(BF16),
        "w1r": np.ascontiguousarray(W1[H + 1:]).astype(BF16),
        "wdt": np.tile(np.ascontiguousarray(W1[H:H + 1]).astype(BF16),
                       (1, IBLK // 2)),
        "rhsb": _rhs_base(),
        "b1": b1.astype(f32),
        "w2": W2.astype(BF16),
        "b2w": np.tile(b2.astype(f32).reshape(D_OUT, 1), (1, 2 * B)),
        "ident": np.eye(128, dtype=BF16),
    }

    for core in range(NCORES):
        enc = core // 4
        s = core % 4
        if enc == 0:
            seqs, lens = inputs["input_queries"], inputs["query_lengths"]
            Wg, bgv, Wc, bcv = (inputs["Wg_q"], inputs["bg_q"],
                                inputs["Wc_q"], inputs["bc_q"])
        else:
            seqs, lens = inputs["input_replies"], inputs["reply_lengths"]
            Wg, bgv, Wc, bcv = (inputs["Wg_r"], inputs["bg_r"],
                                inputs["Wc_r"], inputs["bc_r"])
        rows = slice(BSH * s, BSH * s + BSH)
        xe = emb[seqs[rows]]                       # [64, 40, 256]
        xT = np.transpose(xe, (2, 1, 0)).reshape(E, BT)  # col = t*64+b
        ones_row = np.ones((1, BT), f32)
        lmask = (np.arange(T)[:, None] >= lens[rows][None, :]) \
            .astype(f32).reshape(1, BT)
        xembT = np.concatenate([xT, ones_row, lmask], axis=0).astype(BF16)

        # extra-row weights: row0 multiplies ones (biases), row1 multiplies
        # the length mask (+30 on z-gate preactivation freezes h)
        wxe = np.zeros((2, 3 * H), f32)
        wxe[0, :2 * H] = bgv
        wxe[0, 2 * H:] = bcv
        wxe[1, H:2 * H] = 30.0

        m = {
            "xembT": xembT,
            "whg": np.ascontiguousarray(Wg[E:]).astype(BF16),
            "wxg": np.ascontiguousarray(Wg[:E]).astype(BF16),
            "wch": np.ascontiguousarray(Wc[E:]).astype(BF16),
            "wxc": np.ascontiguousarray(Wc[:E]).astype(BF16),
            "wxe": wxe.astype(BF16),
        }
        m.update(common)
        in_maps.append(m)
    return in_maps


def run_cores(in_maps, trace=False):
    from concourse.bass_utils import run_bass_kernel_spmd
    from concourse.bass_interp import get_hw_module

    if "nc" not in _cache:
        _cache["nc"] = _build()
    nc = _cache["nc"]
    old = nc.m
    nc.m = _cache.setdefault("hwm", get_hw_module(nc.m))
    try:
        res = run_bass_kernel_spmd(nc, in_maps, core_ids=list(range(NCORES)),
                                   trace=trace)
    finally:
        nc.m = old
    return res


def kernel(**inputs):
    in_maps = _prep_inputs(inputs)
    res = run_cores(in_maps)
    logits = np.zeros((B, B, 2), np.float32)
    for core in range(NCORES):
        o = res.results[core]["out"]               # [2, 32*256]
        # pair layout: col = 512*pr + 256*ii + j, local row = 16*ii + pr
        logits[IBLK * core:IBLK * core + IBLK] = \
            o.reshape(2, 16, 2, B).transpose(2, 1, 3, 0).reshape(IBLK, B, 2)
    pos = logits[np.arange(B), np.arange(B)]
    qi, ri = np.nonzero(~np.eye(B, dtype=bool))
    neg = logits[qi, ri]
    return np.concatenate([pos, neg], axis=0).astype(np.float32)


if __name__ == "__main__":
    _build()
    print("build OK")
